# revision 3
# baseline (speedup 1.0000x reference)
"""v6: v4 with high-duty warmup (8x N=512, alternating weights).

Paged sparse-attention kernel for TRN2, head-sharded across 8 NeuronCores.

Structural fact: the mask triu(ones(1024, 5120), 1) means only the first 1024
cached tokens (64 pages) are ever attended, causally.  Per core: 1 kv head,
4 query heads, causal 1024x1024 attention.

v3: single packed input buffer loaded by three ordered DMAs on the sync
HWDGE ring (first-needed bytes first), warmup matmuls bridge the DMA wait
from ~engine-start (gpsimd memset unblocks them immediately), PV chains run
t-outer/head-inner so the final k-tile's exp unblocks only two matmuls per
accumulator at the end.  S^T regions pack 2 heads with the second head's
block column-shifted so each exp is one contiguous valid-only instruction.
Outputs ship unnormalized (bf16 O | denominator); host divides.
"""

import numpy as np
from ml_dtypes import bfloat16

import concourse.bass as bass
import concourse.bacc as bacc
import concourse.mybir as mybir
from concourse.tile import TileContext
from concourse.bass_utils import run_bass_kernel_spmd

NCORES = 8
NUM_HEADS = 32
HPC = NUM_HEADS // NCORES          # 4 query heads per core
D = 128
TQ = 1024
NKT = 8                            # k-token tiles of 128 that survive the mask
SCALE = 0.08838834764831845

# packed input column offsets
OFF_KT = 0
OFF_QC0 = 1024                     # + 512*h
OFF_TRI = 3072
OFF_QC1 = 3200                     # + 512*h
OFF_V16 = 5248                     # + 130*t
INP_W = 6288

f32 = mybir.dt.float32
bf16 = mybir.dt.bfloat16
EXP = mybir.ActivationFunctionType.Exp

_NC_CACHE: list = []


def build_bass() -> bass.Bass:
    nc = bacc.Bacc(None, target_bir_lowering=False)
    inp_ext = nc.declare_dram_parameter("inp", [128, INP_W], bf16, isOutput=False)
    out_ext = nc.declare_dram_parameter("out", [8, 2, 128, 260], bf16, isOutput=True)

    with TileContext(nc) as tc:
        with tc.tile_pool(name="big", bufs=1) as big, \
             tc.tile_pool(name="spsum", bufs=2, space="PSUM") as spsum, \
             tc.tile_pool(name="opsum", bufs=4, space="PSUM") as opsum:

            INP = big.tile([128, INP_W], bf16, name="INP", tag="INP")
            warm = big.tile([128, 640], bf16, name="warm", tag="warm")

            def qbase(h, c):
                return (OFF_QC1 if c else OFF_QC0) + 512 * h

            # ---- input DMAs on the sync HWDGE ring, first-needed first ----
            nc.sync.dma_start(out=INP[:, 0:2048], in_=inp_ext[:, 0:2048])
            nc.sync.dma_start(out=INP[:, 2048:3200], in_=inp_ext[:, 2048:3200])
            nc.sync.dma_start(out=INP[:, 3200:INP_W], in_=inp_ext[:, 3200:INP_W])

            # ---- PE warmup bridges the input-DMA wait and opens the HAM
            # clock gate; gpsimd memset so it starts at engine-start ----
            nc.gpsimd.memset(warm[:], 0.25)
            # N=512 keeps the PE array at high duty so the HAM SHORT window
            # reads busy and unthrottles to 2.4 GHz before the first S^T;
            # alternating weight tiles let LDWEIGHTS overlap the prior MM.
            for w in range(8):
                wp = spsum.tile([128, 1024], f32, tag="st", name=f"wp{w}")
                wsl = slice(128 * (w % 2), 128 * (w % 2) + 128)
                nc.tensor.matmul(wp[:, 0:512], lhsT=warm[:, wsl],
                                 rhs=warm[:, 128:640], start=True, stop=True,
                                 skip_group_check=True)

            PT = {}          # (c, t, hp) -> bf16 tile [128, 1024]
            MOFF = {}        # (c, t) -> column shift of the valid region
            mask_rr = [0]

            def phase1(c, t, hp):
                # S^T block for k-tile t, q-chunk c, heads (2hp, 2hp+1).
                # Head hh=1's block is shifted left by moff so the valid
                # columns [moff : 1024-moff] are contiguous -> single exp.
                m = max(0, t - 4 * c)
                moff = 128 * m
                MOFF[(c, t)] = moff
                st = spsum.tile([128, 1024], f32, tag="st", name=f"st{c}_{t}_{hp}")
                for hh in (0, 1):
                    h = 2 * hp + hh
                    nc.tensor.matmul(
                        st[:, 512 * hh + moff * (1 - hh):512 * (hh + 1) - moff * hh],
                        lhsT=INP[:, OFF_KT + 128 * t:OFF_KT + 128 * t + 128],
                        rhs=INP[:, qbase(h, c) + moff:qbase(h, c) + 512],
                        start=True, stop=True)
                pt = big.tile([128, 1024], bf16, tag=f"pt{c}_{t}_{hp}",
                              name=f"pt{c}_{t}_{hp}")
                PT[(c, t, hp)] = pt
                nc.scalar.activation(pt[:, moff:1024 - moff],
                                     st[:, moff:1024 - moff], EXP, scale=SCALE)
                if t >= 4 * c:
                    # diagonal blocks: zero the lower triangle after exp
                    for hh in (0, 1):
                        dcol = 512 * hh + moff * (1 - hh)
                        eng = (nc.vector, nc.gpsimd)[mask_rr[0] % 2]
                        mask_rr[0] += 1
                        eng.tensor_mul(pt[:, dcol:dcol + 128],
                                       pt[:, dcol:dcol + 128],
                                       INP[:, OFF_TRI:OFF_TRI + 128])

            def pv_part(g, hp, t_lo, t_hi, op):
                # accumulate O[q-sub g, d|1] over k-tiles [t_lo, t_hi] for
                # heads (2hp, 2hp+1), t-outer so the last k-tile's exp gates
                # only two matmuls.  start=True clears has_written for the
                # WHOLE bank, so only the very first matmul into this op
                # tile sets it; the second head's first write overwrites
                # where the bit is unset (fresh-accumulate semantics).
                c, lm = g // 4, g % 4
                for t in range(t_lo, t_hi + 1):
                    moff = MOFF[(c, t)]
                    for hh in (0, 1):
                        col = 512 * hh + 128 * lm - moff * hh
                        nc.tensor.matmul(
                            op[:, 130 * hh:130 * hh + 129],
                            lhsT=PT[(c, t, hp)][:, col:col + 128],
                            rhs=INP[:, OFF_V16 + 130 * t:OFF_V16 + 130 * t + 129],
                            start=(hh == 0 and t == 0 and t_lo == 0),
                            stop=(t == g),
                            skip_group_check=True)

            def pv_finish(g, ops):
                # per-head-pair copy+DMA so the last exp gates only two
                # matmuls, one copy and one small DMA
                for hp in (0, 1):
                    ost = big.tile([128, 260], bf16, tag="ost", bufs=4,
                                   name=f"ost{g}_{hp}")
                    nc.vector.tensor_copy(ost[:], ops[hp][:])
                    nc.sync.dma_start(out=out_ext[g, hp], in_=ost[:])

            def new_op(g, hp):
                return opsum.tile([128, 260], f32, tag="op", name=f"op{g}_{hp}")

            # ---- program order ----
            for t in range(4):                   # S^T chunk0 (diagonal tiles)
                for hp in (0, 1):
                    phase1(0, t, hp)
            for t in range(4):                   # S^T chunk1, full tiles
                for hp in (0, 1):
                    phase1(1, t, hp)
            for g in range(4):                   # PV chunk0
                ops = [new_op(g, hp) for hp in (0, 1)]
                for hp in (0, 1):
                    pv_part(g, hp, 0, g, ops[hp])
                pv_finish(g, ops)
            # PV prefixes for g=4,5 over the already-exp'd full tiles
            ops45 = {}
            for g in (4, 5):
                ops45[g] = [new_op(g, hp) for hp in (0, 1)]
                for hp in (0, 1):
                    pv_part(g, hp, 0, 3, ops45[g][hp])
            for t in range(4, 8):                # S^T chunk1 diagonal tiles
                for hp in (0, 1):
                    phase1(1, t, hp)
                g = t
                if g in (4, 5):
                    for hp in (0, 1):
                        pv_part(g, hp, 4, g, ops45[g][hp])
                    pv_finish(g, ops45[g])
            for g in (6, 7):
                ops = [new_op(g, hp) for hp in (0, 1)]
                for hp in (0, 1):
                    pv_part(g, hp, 0, g, ops[hp])
                pv_finish(g, ops)

    return nc


def _get_nc() -> bass.Bass:
    if not _NC_CACHE:
        nc = build_bass()
        nc.finalize()
        _NC_CACHE.append(nc)
    return _NC_CACHE[0]


def run(q, k, v, kv_cache, block_table, trace=False):
    q = np.asarray(q, dtype=np.float32)
    kv_cache = np.asarray(kv_cache, dtype=np.float32)
    bt64 = np.asarray(block_table).astype(np.int64)[:64]
    nc = _get_nc()

    tri = np.triu(np.ones((128, 128), np.float32))
    pages = kv_cache[bt64]                        # [64, 2, 8, 16, 128]
    qq = q.reshape(TQ, NUM_HEADS, D)

    in_maps = []
    for i in range(NCORES):
        kg = pages[:, 0, i].reshape(TQ, D)        # [k_tok, d]
        vg = pages[:, 1, i].reshape(NKT, 128, D)  # [t, k_in_tile, d]
        inp = np.zeros((128, INP_W), np.float32)
        inp[:, OFF_KT:OFF_KT + TQ] = kg.T
        qsub = qq[:, 4 * i:4 * i + 4, :]          # [q, h, d]
        for h in range(HPC):
            inp[:, OFF_QC0 + 512 * h:OFF_QC0 + 512 * h + 512] = qsub[0:512, h].T
            inp[:, OFF_QC1 + 512 * h:OFF_QC1 + 512 * h + 512] = qsub[512:1024, h].T
        inp[:, OFF_TRI:OFF_TRI + 128] = tri
        for t in range(NKT):
            inp[:, OFF_V16 + 130 * t:OFF_V16 + 130 * t + 128] = vg[t]
            inp[:, OFF_V16 + 130 * t + 128] = 1.0
        in_maps.append({"inp": inp.astype(bfloat16)})
    res = run_bass_kernel_spmd(nc, in_maps, list(range(NCORES)), trace=trace)

    out = np.empty((TQ, NUM_HEADS * D), np.float32)
    for i in range(NCORES):
        raw = np.asarray(res.results[i]["out"]).astype(np.float32)
        blk = raw.reshape(8, 2, 128, 2, 130).transpose(0, 2, 1, 3, 4)
        blk = blk.reshape(8, 128, 4, 130)         # [g, q_local, h, d|den|pad]
        o = blk[..., :128] / blk[..., 128:129]    # normalize on host
        out[:, 512 * i:512 * (i + 1)] = o.reshape(TQ, HPC * D)
    return out, res


def kernel(q, k, v, kv_cache, block_table):
    out, _ = run(q, k, v, kv_cache, block_table, trace=False)
    return out


# revision 4
# speedup vs baseline: 1.1640x; 1.1640x over previous
"""v9: v8 + exactly-packed chunk-0 exp regions (8 -> 5 instrs).

Paged sparse-attention kernel for TRN2, head-sharded across 8 NeuronCores.

Structural fact: the mask triu(ones(1024, 5120), 1) means only the first 1024
cached tokens (64 pages) are ever attended, causally.  Per core: 1 kv head,
4 query heads, causal 1024x1024 attention.

v3: single packed input buffer loaded by three ordered DMAs on the sync
HWDGE ring (first-needed bytes first), warmup matmuls bridge the DMA wait
from ~engine-start (gpsimd memset unblocks them immediately), PV chains run
t-outer/head-inner so the final k-tile's exp unblocks only two matmuls per
accumulator at the end.  S^T regions pack 2 heads with the second head's
block column-shifted so each exp is one contiguous valid-only instruction.
Outputs ship unnormalized (bf16 O | denominator); host divides.
"""

import numpy as np
from ml_dtypes import bfloat16

import concourse.bass as bass
import concourse.bacc as bacc
import concourse.mybir as mybir
from concourse.tile import TileContext
from concourse.bass_utils import run_bass_kernel_spmd

NCORES = 8
NUM_HEADS = 32
HPC = NUM_HEADS // NCORES          # 4 query heads per core
D = 128
TQ = 1024
NKT = 8                            # k-token tiles of 128 that survive the mask
SCALE = 0.08838834764831845

# packed input column offsets
OFF_KT = 0
OFF_QC0 = 1024                     # + 512*h
OFF_TRI = 3072
OFF_QC1 = 3200                     # + 512*h
OFF_V16 = 5248                     # + 130*t
INP_W = 6288


def ktcol(t):
    return OFF_KT + 128 * t

f32 = mybir.dt.float32
bf16 = mybir.dt.bfloat16
EXP = mybir.ActivationFunctionType.Exp

_NC_CACHE: list = []


def build_bass() -> bass.Bass:
    nc = bacc.Bacc(None, target_bir_lowering=False)
    inp_ext = nc.declare_dram_parameter("inp", [128, INP_W], bf16, isOutput=False)
    out_ext = nc.declare_dram_parameter("out", [8, 2, 128, 260], bf16, isOutput=True)

    with TileContext(nc) as tc:
        with tc.tile_pool(name="big", bufs=1) as big, \
             tc.tile_pool(name="spsum", bufs=2, space="PSUM") as spsum, \
             tc.tile_pool(name="opsum", bufs=4, space="PSUM") as opsum:

            INP = big.tile([128, INP_W], bf16, name="INP", tag="INP")
            warm = big.tile([128, 512], bf16, name="warm", tag="warm")

            def qbase(h, c):
                return (OFF_QC1 if c else OFF_QC0) + 512 * h

            # ---- input DMAs on the sync HWDGE ring, first-needed first ----
            nc.sync.dma_start(out=INP[:, 0:2048], in_=inp_ext[:, 0:2048])
            nc.sync.dma_start(out=INP[:, 2048:3200], in_=inp_ext[:, 2048:3200])
            nc.sync.dma_start(out=INP[:, 3200:INP_W], in_=inp_ext[:, 3200:INP_W])

            # ---- PE warmup bridges the input-DMA wait and opens the HAM
            # clock gate; gpsimd memset so it starts at engine-start ----
            nc.gpsimd.memset(warm[:], 0.25)
            # N=512 keeps the PE array at high duty so the HAM SHORT window
            # reads busy and unthrottles to 2.4 GHz before the first S^T;
            # alternating weight tiles let LDWEIGHTS overlap the prior MM.
            for w in range(8):
                wp = spsum.tile([128, 1024], f32, tag="st", name=f"wp{w}")
                wsl = slice(128 * (w % 2), 128 * (w % 2) + 128)
                nc.tensor.matmul(wp[:, 0:512], lhsT=warm[:, wsl],
                                 rhs=warm[:, 0:512], start=True, stop=True,
                                 skip_group_check=True)

            PTLOC = {}       # (c, t, h) -> (pt_tile, pack_col)
            MOFF = {}        # (c, t) -> column shift of the valid region
            mask_rr = [0]

            def region(c, blocks, rname):
                # One S^T region [128, 1024] = one exp instruction.  blocks =
                # [(h, t, pack_col), ...], each bank-aligned; a block covers
                # q-local [moff : 512] placed at pack_col; the [0:1024] span
                # is exactly covered by valid columns.
                st = spsum.tile([128, 1024], f32, tag="st", name=f"st{rname}")
                for h, t, pcol in blocks:
                    m = max(0, t - 4 * c)
                    moff = 128 * m
                    MOFF[(c, t)] = moff
                    nc.tensor.matmul(
                        st[:, pcol:pcol + 512 - moff],
                        lhsT=INP[:, ktcol(t):ktcol(t) + 128],
                        rhs=INP[:, qbase(h, c) + moff:qbase(h, c) + 512],
                        start=True, stop=True)
                pt = big.tile([128, 1024], bf16, tag=f"pt{rname}",
                              name=f"pt{rname}")
                lo = min(p for _, _, p in blocks)
                hi = max(p + 512 - 128 * max(0, t - 4 * c)
                         for _, t, p in blocks)
                nc.scalar.activation(pt[:, lo:hi], st[:, lo:hi], EXP,
                                     scale=SCALE)
                for h, t, pcol in blocks:
                    PTLOC[(c, t, h)] = (pt, pcol)
                    if t >= 4 * c:
                        # diagonal block: zero the lower triangle after exp
                        eng = (nc.vector, nc.gpsimd)[mask_rr[0] % 2]
                        mask_rr[0] += 1
                        eng.tensor_mul(pt[:, pcol:pcol + 128],
                                       pt[:, pcol:pcol + 128],
                                       INP[:, OFF_TRI:OFF_TRI + 128])

            def phase1(c, t, hp):
                # classic 2-head region for chunk-1 tiles
                moff = 128 * max(0, t - 4 * c)
                region(c, [(2 * hp, t, moff), (2 * hp + 1, t, 512)],
                       f"{c}_{t}_{hp}")

            def pv_part(g, hp, t_lo, t_hi, op):
                # accumulate O[q-sub g, d|1] over k-tiles [t_lo, t_hi] for
                # heads (2hp, 2hp+1), t-outer so the last k-tile's exp gates
                # only two matmuls.  start=True clears has_written for the
                # WHOLE bank, so only the very first matmul into this op
                # tile sets it; the second head's first write overwrites
                # where the bit is unset (fresh-accumulate semantics).
                c, lm = g // 4, g % 4
                for t in range(t_lo, t_hi + 1):
                    moff = MOFF[(c, t)]
                    for hh in (0, 1):
                        pt, pcol = PTLOC[(c, t, 2 * hp + hh)]
                        col = pcol + 128 * lm - moff
                        nc.tensor.matmul(
                            op[:, 130 * hh:130 * hh + 129],
                            lhsT=pt[:, col:col + 128],
                            rhs=INP[:, OFF_V16 + 130 * t:OFF_V16 + 130 * t + 129],
                            start=(hh == 0 and t == 0 and t_lo == 0),
                            stop=(t == g),
                            skip_group_check=True)

            def pv_finish(g, ops):
                # per-head-pair copy+DMA so the last exp gates only two
                # matmuls, one copy and one small DMA
                for hp in (0, 1):
                    ost = big.tile([128, 260], bf16, tag="ost", bufs=4,
                                   name=f"ost{g}_{hp}")
                    nc.vector.tensor_copy(ost[:], ops[hp][:])
                    nc.sync.dma_start(out=out_ext[g, hp], in_=ost[:])

            def new_op(g, hp):
                return opsum.tile([128, 260], f32, tag="op", name=f"op{g}_{hp}")

            # ---- program order ----
            # chunk0: five exactly-packed regions (all blocks diagonal);
            # the first two need only the first input DMA (heads 0,1)
            region(0, [(0, 0, 0), (1, 0, 512)], "c0A")
            region(0, [(0, 1, 0), (0, 3, 384), (1, 1, 512), (1, 3, 896)], "c0B")
            region(0, [(2, 0, 0), (3, 0, 512)], "c0C")
            region(0, [(2, 1, 0), (2, 3, 384), (3, 1, 512), (3, 3, 896)], "c0D")
            region(0, [(0, 2, 0), (1, 2, 256), (2, 2, 512), (3, 2, 768)], "c0E")
            for t in range(4):                   # S^T chunk1, full tiles
                for hp in (0, 1):
                    phase1(1, t, hp)
            for g in range(4):                   # PV chunk0
                ops = [new_op(g, hp) for hp in (0, 1)]
                for hp in (0, 1):
                    pv_part(g, hp, 0, g, ops[hp])
                pv_finish(g, ops)
            # PV prefixes for g=4,5 over the already-exp'd full tiles
            ops45 = {}
            for g in (4, 5):
                ops45[g] = [new_op(g, hp) for hp in (0, 1)]
                for hp in (0, 1):
                    pv_part(g, hp, 0, 3, ops45[g][hp])
            for t in range(4, 8):                # S^T chunk1 diagonal tiles
                for hp in (0, 1):
                    phase1(1, t, hp)
                g = t
                if g in (4, 5):
                    for hp in (0, 1):
                        pv_part(g, hp, 4, g, ops45[g][hp])
                    pv_finish(g, ops45[g])
            for g in (6, 7):
                ops = [new_op(g, hp) for hp in (0, 1)]
                for hp in (0, 1):
                    pv_part(g, hp, 0, g, ops[hp])
                pv_finish(g, ops)

    return nc


def _get_nc() -> bass.Bass:
    if not _NC_CACHE:
        nc = build_bass()
        nc.finalize()
        _NC_CACHE.append(nc)
    return _NC_CACHE[0]


def run(q, k, v, kv_cache, block_table, trace=False):
    q = np.asarray(q, dtype=np.float32)
    kv_cache = np.asarray(kv_cache, dtype=np.float32)
    bt64 = np.asarray(block_table).astype(np.int64)[:64]
    nc = _get_nc()

    tri = np.triu(np.ones((128, 128), np.float32))
    pages = kv_cache[bt64]                        # [64, 2, 8, 16, 128]
    qq = q.reshape(TQ, NUM_HEADS, D)

    in_maps = []
    for i in range(NCORES):
        kg = pages[:, 0, i].reshape(TQ, D)        # [k_tok, d]
        vg = pages[:, 1, i].reshape(NKT, 128, D)  # [t, k_in_tile, d]
        inp = np.zeros((128, INP_W), np.float32)
        inp[:, OFF_KT:OFF_KT + TQ] = kg.T
        qsub = qq[:, 4 * i:4 * i + 4, :]          # [q, h, d]
        for h in range(HPC):
            inp[:, OFF_QC0 + 512 * h:OFF_QC0 + 512 * h + 512] = qsub[0:512, h].T
            inp[:, OFF_QC1 + 512 * h:OFF_QC1 + 512 * h + 512] = qsub[512:1024, h].T
        inp[:, OFF_TRI:OFF_TRI + 128] = tri
        for t in range(NKT):
            inp[:, OFF_V16 + 130 * t:OFF_V16 + 130 * t + 128] = vg[t]
            inp[:, OFF_V16 + 130 * t + 128] = 1.0
        in_maps.append({"inp": inp.astype(bfloat16)})
    res = run_bass_kernel_spmd(nc, in_maps, list(range(NCORES)), trace=trace)

    out = np.empty((TQ, NUM_HEADS * D), np.float32)
    for i in range(NCORES):
        raw = np.asarray(res.results[i]["out"]).astype(np.float32)
        blk = raw.reshape(8, 2, 128, 2, 130).transpose(0, 2, 1, 3, 4)
        blk = blk.reshape(8, 128, 4, 130)         # [g, q_local, h, d|den|pad]
        o = blk[..., :128] / blk[..., 128:129]    # normalize on host
        out[:, 512 * i:512 * (i + 1)] = o.reshape(TQ, HPC * D)
    return out, res


def kernel(q, k, v, kv_cache, block_table):
    out, _ = run(q, k, v, kv_cache, block_table, trace=False)
    return out
